# revision 15
# baseline (speedup 1.0000x reference)
"""Trainium2 Bass kernel for nn_PredictionDecoder.

Factorization: the reference materializes embed [B,N,D] but immediately
contracts it with w_st, so

  out[b,n] = (1 - beta[b,n]*alpha[n]) * s[n]
             + scatter corrections at <= 2*B*K positions
             + user_out[b] + b_st
  with s[n] = (station_emb_table[n] @ proj_w + proj_b) @ w_st
            = station_emb_table[n] @ v + c,   v = proj_w @ w_st

The memory-bound part is the [N,D] @ [D] matvec (streams the 30.8 MB
table).  N is sharded over the 8 cores.  The fp32 table is shipped as a
bf16 hi/lo split (same bytes, ~2^-18 relative precision) because bf16
matmuls get fast weight load; each 128-row chunk is the stationary
operand and v (replicated x8 for the batch dim, hi and lo) is the moving
operand, so PSUM directly holds the [n_part, b] output layout.  The
vector engine sums the three product terms plus the per-b bias, and the
[7680, 8] output slice leaves with fully-contiguous DMA.  The <=512
beta/scatter-affected (b, n) entries are recomputed exactly on the host
and patched in.
"""

import os
import numpy as np

B = 8
D = 128
N_USER = 60082
NCORES = 8
MC = 7680                 # table rows per core (padded)
NPAD = NCORES * MC        # 61440
TBLK = 3840               # columns per DMA block
NBLK = MC // TBLK         # 2
KPB = TBLK // 128         # 30 matmul chunks per block
NCHUNK = MC // 128        # 60
FREE = NCHUNK * B         # 480 output columns
PFREE = NCHUNK * 24       # 1440 psum columns (16 hi-terms + 8 lo-term)

_CACHE = {}
LAST_RESULT = None


def _build_program():
    import concourse.tile as tile
    from concourse import bacc, mybir

    nc = bacc.Bacc("TRN2", target_bir_lowering=False, debug=False,
                   num_devices=NCORES)
    f32 = mybir.dt.float32
    bf16 = mybir.dt.bfloat16
    tabhi = nc.dram_tensor("tabhi", [128, MC], bf16, kind="ExternalInput")
    tablo = nc.dram_tensor("tablo", [128, MC], bf16, kind="ExternalInput")
    v16 = nc.dram_tensor("v16", [128, 2 * B], bf16, kind="ExternalInput")
    urep = nc.dram_tensor("urep", [128, FREE], f32, kind="ExternalInput")
    res = nc.dram_tensor("res", [MC, B], f32, kind="ExternalOutput")

    with tile.TileContext(nc) as tc:
        with tc.tile_pool(name="hi", bufs=NBLK) as hi_pool, \
             tc.tile_pool(name="lo", bufs=NBLK) as lo_pool, \
             tc.tile_pool(name="small", bufs=1) as small_pool, \
             tc.tile_pool(name="outp", bufs=1) as out_pool, \
             tc.tile_pool(name="ps", bufs=1, space="PSUM") as ps_pool:
            v16_sb = small_pool.tile([128, 2 * B], bf16, tag="v16")
            nc.gpsimd.dma_start(v16_sb[:], v16.ap())
            u_sb = small_pool.tile([128, FREE], f32, tag="u")
            nc.gpsimd.dma_start(u_sb[:], urep.ap())

            psum_hi = ps_pool.tile([128, NCHUNK * 16], f32, tag="ph")
            psum_lo = ps_pool.tile([128, NCHUNK * 8], f32, tag="pl")
            out_sb = out_pool.tile([128, FREE], f32, tag="o")
            tmp1 = out_pool.tile([128, FREE], f32, tag="t1")
            tmp2 = out_pool.tile([128, FREE], f32, tag="t2")

            # warm-up matmul consuming only v16 so the PE observes the v16 DMA
            # semaphore before the table matmuls begin.
            scr = ps_pool.tile([2 * B, 2 * B], f32, tag="scr")
            nc.tensor.matmul(out=scr[:], lhsT=v16_sb[:], rhs=v16_sb[:],
                             start=True, stop=True)

            for t in range(NBLK):
                sl = slice(TBLK * t, TBLK * (t + 1))
                hi_blk = hi_pool.tile([128, TBLK], bf16, tag="hi")
                nc.sync.dma_start(hi_blk[:], tabhi.ap()[:, sl])
                lo_blk = lo_pool.tile([128, TBLK], bf16, tag="lo")
                nc.scalar.dma_start(lo_blk[:], tablo.ap()[:, sl])
                for kk in range(KPB):
                    k = KPB * t + kk
                    wsl = slice(128 * kk, 128 * (kk + 1))
                    # hi @ [v_hi | v_lo] -> 16 cols
                    nc.tensor.matmul(
                        out=psum_hi[:, 16 * k:16 * k + 16],
                        lhsT=hi_blk[:, wsl], rhs=v16_sb[:],
                        start=True, stop=True)
                    # lo @ v_hi -> 8 cols
                    nc.tensor.matmul(
                        out=psum_lo[:, 8 * k:8 * k + 8],
                        lhsT=lo_blk[:, wsl], rhs=v16_sb[:, 0:B],
                        start=True, stop=True)

            pv = psum_hi[:].rearrange("p (k c) -> p k c", c=16)
            pl = psum_lo[:].rearrange("p (k c) -> p k c", c=B)
            o3 = out_sb[:].rearrange("p (k c) -> p k c", c=B)
            t13 = tmp1[:].rearrange("p (k c) -> p k c", c=B)
            t23 = tmp2[:].rearrange("p (k c) -> p k c", c=B)
            u3 = u_sb[:].rearrange("p (k c) -> p k c", c=B)
            resv = res.ap().rearrange("(p q) b -> p (q b)", p=128)
            # tail in halves so adds/out-DMA overlap the last input blocks;
            # DVE may read only one PSUM operand per instruction
            for h in range(2):
                ck = slice(NCHUNK // 2 * h, NCHUNK // 2 * (h + 1))
                fs = slice(FREE // 2 * h, FREE // 2 * (h + 1))
                nc.vector.tensor_add(t13[:, ck], pv[:, ck, 0:8], u3[:, ck])
                nc.vector.tensor_add(t23[:, ck], t13[:, ck], pv[:, ck, 8:16])
                nc.vector.tensor_add(o3[:, ck], t23[:, ck], pl[:, ck])
                nc.gpsimd.dma_start(resv[:, fs], out_sb[:, fs])
    nc.compile()
    return nc


def _get_program():
    if "nc" not in _CACHE:
        _CACHE["nc"] = _build_program()
    return _CACHE["nc"]


def _leaky_relu(x):
    return np.where(x >= 0, x, 0.01 * x)


def _hi_lo(x, bf16):
    hi = x.astype(bf16)
    lo = (x - hi.astype(np.float32)).astype(bf16)
    return hi, lo


def kernel(user_embedding, station_embedding, raw_field_embed,
           user_emb_table, station_emb_table, proj_w, proj_b,
           theta, alpha, w_his1, b_his1, w_his2, b_his2,
           w_st, b_st, w_u, b_u,
           his_nodes, now_nodes, user_id):
    import ml_dtypes
    from concourse.bass_utils import run_bass_kernel_spmd
    global LAST_RESULT

    f32 = np.float32
    bf16 = ml_dtypes.bfloat16
    user_embedding = np.asarray(user_embedding, f32)
    station_embedding = np.asarray(station_embedding, f32)
    raw_field_embed = np.asarray(raw_field_embed, f32)
    user_emb_table = np.asarray(user_emb_table, f32)
    station_emb_table = np.asarray(station_emb_table, f32)
    proj_w = np.asarray(proj_w, f32)
    proj_b = np.asarray(proj_b, f32).reshape(-1)
    theta = np.asarray(theta, f32)
    alpha = np.asarray(alpha, f32)
    w_his1 = np.asarray(w_his1, f32)
    b_his1 = np.asarray(b_his1, f32).reshape(-1)
    w_his2 = np.asarray(w_his2, f32)
    b_his2 = np.asarray(b_his2, f32).reshape(-1)
    w_st = np.asarray(w_st, f32).reshape(-1)
    b_st = np.asarray(b_st, f32).reshape(-1)
    w_u = np.asarray(w_u, f32).reshape(-1)
    b_u = np.asarray(b_u, f32).reshape(-1)
    his_nodes = np.asarray(his_nodes).astype(np.int64)
    now_nodes = np.asarray(now_nodes).astype(np.int64)
    user_id = np.asarray(user_id).astype(np.int64)

    n_users = station_emb_table.shape[0]

    # host-side small linear algebra (exact, matches reference math)
    v = proj_w @ w_st                                   # [D]
    c = f32(proj_b @ w_st)                              # scalar
    th = theta[user_id, 0]                              # [B]
    user_mem = (1.0 - th)[:, None] * user_embedding \
        + th[:, None] * user_emb_table[user_id]         # [B, D]
    user_out = user_mem @ w_u + b_u[0]                  # [B]
    bias = (user_out + b_st[0] + c).astype(f32)         # [B]

    # device inputs
    tab_pad = np.zeros((NPAD, D), f32)
    tab_pad[:n_users] = station_emb_table
    v_hi, v_lo = _hi_lo(v.astype(f32), bf16)
    v16 = np.empty((D, 2 * B), bf16)
    v16[:, 0:B] = v_hi[:, None]
    v16[:, B:2 * B] = v_lo[:, None]
    urep = np.ascontiguousarray(
        np.broadcast_to(np.tile(bias, NCHUNK)[None, :], (128, FREE)))

    in_maps = []
    for core in range(NCORES):
        shard = np.ascontiguousarray(
            tab_pad[MC * core:MC * (core + 1)].T)       # [128, MC]
        hi, lo = _hi_lo(shard, bf16)
        in_maps.append({"tabhi": hi, "tablo": lo, "v16": v16, "urep": urep})

    nc = _get_program()
    trace = bool(int(os.environ.get("KERNEL_TRACE", "0")))
    LAST_RESULT = run_bass_kernel_spmd(
        nc, in_maps, core_ids=list(range(NCORES)), trace=trace)

    # unshard: res[60*j + 6*t + kk, b] = s[off + 768*t + 128*kk + j] + bias[b]
    out = np.empty((B, N_USER), f32)
    for core in range(NCORES):
        res = LAST_RESULT.results[core]["res"]          # [MC, B]
        cols = res.reshape(128, NBLK, KPB, B).transpose(1, 2, 0, 3) \
            .reshape(MC, B)                             # [MC(station-major), B]
        lo_ = MC * core
        hi_ = min(MC * (core + 1), n_users)
        out[:, lo_:hi_] = cols[:hi_ - lo_].T

    # exact host recomputation of the <=2*B*K beta/scatter-affected entries
    for b in range(B):
        now_b = now_nodes[b]
        his_b = his_nodes[b]
        aff = np.unique(np.concatenate([now_b, his_b]))
        pos_now = np.searchsorted(aff, now_b)
        pos_his = np.searchsorted(aff, his_b)

        proj_aff = station_emb_table[aff] @ proj_w + proj_b     # [m, D]
        embed = (1.0 - alpha[aff]) * proj_aff                   # beta == 1
        now_add = alpha[now_b] * station_embedding[now_b]       # [K, D]
        np.add.at(embed, pos_now, now_add)
        h = _leaky_relu(raw_field_embed[his_b] @ w_his1 + b_his1) \
            @ w_his2 + b_his2                                   # [K, D]
        np.add.at(embed, pos_his, alpha[his_b] * h)
        out[b, aff] = embed @ w_st + b_st[0] + user_out[b]

    return out


# revision 16
# speedup vs baseline: 1.0071x; 1.0071x over previous
"""Trainium2 Bass kernel for nn_PredictionDecoder.

Factorization: the reference materializes embed [B,N,D] but immediately
contracts it with w_st, so

  out[b,n] = (1 - beta[b,n]*alpha[n]) * s[n]
             + scatter corrections at <= 2*B*K positions
             + user_out[b] + b_st
  with s[n] = (station_emb_table[n] @ proj_w + proj_b) @ w_st
            = station_emb_table[n] @ v + c,   v = proj_w @ w_st

The memory-bound part is the [N,D] @ [D] matvec (streams the 30.8 MB
table).  N is sharded over the 8 cores.  The fp32 table is shipped as a
bf16 hi/lo split (same bytes, ~2^-18 relative precision) because bf16
matmuls get fast weight load; each 128-row chunk is the stationary
operand and v (replicated x8 for the batch dim, hi and lo) is the moving
operand, so PSUM directly holds the [n_part, b] output layout.  The
vector engine sums the three product terms plus the per-b bias, and the
[7680, 8] output slice leaves with fully-contiguous DMA.  The <=512
beta/scatter-affected (b, n) entries are recomputed exactly on the host
and patched in.
"""

import os
import numpy as np

B = 8
D = 128
N_USER = 60082
NCORES = 8
MC = 7680                 # table rows per core (padded)
NPAD = NCORES * MC        # 61440
TBLK = 3840               # columns per DMA block
NBLK = MC // TBLK         # 2
KPB = TBLK // 128         # 30 matmul chunks per block
NCHUNK = MC // 128        # 60
FREE = NCHUNK * B         # 480 output columns
PFREE = NCHUNK * 24       # 1440 psum columns (16 hi-terms + 8 lo-term)

_CACHE = {}
LAST_RESULT = None


def _build_program():
    import concourse.tile as tile
    from concourse import bacc, mybir

    nc = bacc.Bacc("TRN2", target_bir_lowering=False, debug=False,
                   num_devices=NCORES)
    f32 = mybir.dt.float32
    bf16 = mybir.dt.bfloat16
    tabhi = nc.dram_tensor("tabhi", [128, MC], bf16, kind="ExternalInput")
    tablo = nc.dram_tensor("tablo", [128, MC], bf16, kind="ExternalInput")
    v16 = nc.dram_tensor("v16", [128, 2 * B], bf16, kind="ExternalInput")
    urep = nc.dram_tensor("urep", [128, FREE], f32, kind="ExternalInput")
    res = nc.dram_tensor("res", [MC, B], f32, kind="ExternalOutput")

    with tile.TileContext(nc) as tc:
        with tc.tile_pool(name="hi", bufs=NBLK) as hi_pool, \
             tc.tile_pool(name="lo", bufs=NBLK) as lo_pool, \
             tc.tile_pool(name="small", bufs=1) as small_pool, \
             tc.tile_pool(name="outp", bufs=1) as out_pool, \
             tc.tile_pool(name="ps", bufs=1, space="PSUM") as ps_pool:
            v16_sb = small_pool.tile([128, 2 * B], bf16, tag="v16")
            nc.gpsimd.dma_start(v16_sb[:], v16.ap())
            u_sb = small_pool.tile([128, FREE], f32, tag="u")
            nc.gpsimd.dma_start(u_sb[:], urep.ap())

            psum_hi = ps_pool.tile([128, NCHUNK * 16], f32, tag="ph")
            psum_lo = ps_pool.tile([128, NCHUNK * 8], f32, tag="pl")
            out_sb = out_pool.tile([128, FREE], f32, tag="o")
            tmp1 = out_pool.tile([128, FREE], f32, tag="t1")
            tmp2 = out_pool.tile([128, FREE], f32, tag="t2")

            # warm-up matmul consuming only v16 so the PE observes the v16 DMA
            # semaphore before the table matmuls begin.
            scr = ps_pool.tile([2 * B, 2 * B], f32, tag="scr")
            nc.tensor.matmul(out=scr[:], lhsT=v16_sb[:], rhs=v16_sb[:],
                             start=True, stop=True)

            for t in range(NBLK):
                sl = slice(TBLK * t, TBLK * (t + 1))
                hi_blk = hi_pool.tile([128, TBLK], bf16, tag="hi")
                nc.sync.dma_start(hi_blk[:], tabhi.ap()[:, sl])
                lo_blk = lo_pool.tile([128, TBLK], bf16, tag="lo")
                nc.scalar.dma_start(lo_blk[:], tablo.ap()[:, sl])
                for kk in range(KPB):
                    k = KPB * t + kk
                    wsl = slice(128 * kk, 128 * (kk + 1))
                    # hi @ [v_hi | v_lo] -> 16 cols
                    nc.tensor.matmul(
                        out=psum_hi[:, 16 * k:16 * k + 16],
                        lhsT=hi_blk[:, wsl], rhs=v16_sb[:],
                        start=True, stop=True)
                    # lo @ v_hi -> 8 cols
                    nc.tensor.matmul(
                        out=psum_lo[:, 8 * k:8 * k + 8],
                        lhsT=lo_blk[:, wsl], rhs=v16_sb[:, 0:B],
                        start=True, stop=True)

            pv = psum_hi[:].rearrange("p (k c) -> p k c", c=16)
            pl = psum_lo[:].rearrange("p (k c) -> p k c", c=B)
            o3 = out_sb[:].rearrange("p (k c) -> p k c", c=B)
            t13 = tmp1[:].rearrange("p (k c) -> p k c", c=B)
            t23 = tmp2[:].rearrange("p (k c) -> p k c", c=B)
            u3 = u_sb[:].rearrange("p (k c) -> p k c", c=B)
            resv = res.ap().rearrange("(p q) b -> p (q b)", p=128)
            # tail in halves so adds/out-DMA overlap the last input blocks;
            # DVE may read only one PSUM operand per instruction
            for h in range(2):
                ck = slice(NCHUNK // 2 * h, NCHUNK // 2 * (h + 1))
                fs = slice(FREE // 2 * h, FREE // 2 * (h + 1))
                nc.vector.tensor_add(t13[:, ck], pv[:, ck, 0:8], u3[:, ck])
                nc.vector.tensor_add(t23[:, ck], t13[:, ck], pv[:, ck, 8:16])
                nc.vector.tensor_add(o3[:, ck], t23[:, ck], pl[:, ck])
                nc.sync.dma_start(resv[:, fs], out_sb[:, fs])
    nc.compile()
    return nc


def _get_program():
    if "nc" not in _CACHE:
        _CACHE["nc"] = _build_program()
    return _CACHE["nc"]


def _leaky_relu(x):
    return np.where(x >= 0, x, 0.01 * x)


def _hi_lo(x, bf16):
    hi = x.astype(bf16)
    lo = (x - hi.astype(np.float32)).astype(bf16)
    return hi, lo


def kernel(user_embedding, station_embedding, raw_field_embed,
           user_emb_table, station_emb_table, proj_w, proj_b,
           theta, alpha, w_his1, b_his1, w_his2, b_his2,
           w_st, b_st, w_u, b_u,
           his_nodes, now_nodes, user_id):
    import ml_dtypes
    from concourse.bass_utils import run_bass_kernel_spmd
    global LAST_RESULT

    f32 = np.float32
    bf16 = ml_dtypes.bfloat16
    user_embedding = np.asarray(user_embedding, f32)
    station_embedding = np.asarray(station_embedding, f32)
    raw_field_embed = np.asarray(raw_field_embed, f32)
    user_emb_table = np.asarray(user_emb_table, f32)
    station_emb_table = np.asarray(station_emb_table, f32)
    proj_w = np.asarray(proj_w, f32)
    proj_b = np.asarray(proj_b, f32).reshape(-1)
    theta = np.asarray(theta, f32)
    alpha = np.asarray(alpha, f32)
    w_his1 = np.asarray(w_his1, f32)
    b_his1 = np.asarray(b_his1, f32).reshape(-1)
    w_his2 = np.asarray(w_his2, f32)
    b_his2 = np.asarray(b_his2, f32).reshape(-1)
    w_st = np.asarray(w_st, f32).reshape(-1)
    b_st = np.asarray(b_st, f32).reshape(-1)
    w_u = np.asarray(w_u, f32).reshape(-1)
    b_u = np.asarray(b_u, f32).reshape(-1)
    his_nodes = np.asarray(his_nodes).astype(np.int64)
    now_nodes = np.asarray(now_nodes).astype(np.int64)
    user_id = np.asarray(user_id).astype(np.int64)

    n_users = station_emb_table.shape[0]

    # host-side small linear algebra (exact, matches reference math)
    v = proj_w @ w_st                                   # [D]
    c = f32(proj_b @ w_st)                              # scalar
    th = theta[user_id, 0]                              # [B]
    user_mem = (1.0 - th)[:, None] * user_embedding \
        + th[:, None] * user_emb_table[user_id]         # [B, D]
    user_out = user_mem @ w_u + b_u[0]                  # [B]
    bias = (user_out + b_st[0] + c).astype(f32)         # [B]

    # device inputs
    tab_pad = np.zeros((NPAD, D), f32)
    tab_pad[:n_users] = station_emb_table
    v_hi, v_lo = _hi_lo(v.astype(f32), bf16)
    v16 = np.empty((D, 2 * B), bf16)
    v16[:, 0:B] = v_hi[:, None]
    v16[:, B:2 * B] = v_lo[:, None]
    urep = np.ascontiguousarray(
        np.broadcast_to(np.tile(bias, NCHUNK)[None, :], (128, FREE)))

    in_maps = []
    for core in range(NCORES):
        shard = np.ascontiguousarray(
            tab_pad[MC * core:MC * (core + 1)].T)       # [128, MC]
        hi, lo = _hi_lo(shard, bf16)
        in_maps.append({"tabhi": hi, "tablo": lo, "v16": v16, "urep": urep})

    nc = _get_program()
    trace = bool(int(os.environ.get("KERNEL_TRACE", "0")))
    LAST_RESULT = run_bass_kernel_spmd(
        nc, in_maps, core_ids=list(range(NCORES)), trace=trace)

    # unshard: res[60*j + 6*t + kk, b] = s[off + 768*t + 128*kk + j] + bias[b]
    out = np.empty((B, N_USER), f32)
    for core in range(NCORES):
        res = LAST_RESULT.results[core]["res"]          # [MC, B]
        cols = res.reshape(128, NBLK, KPB, B).transpose(1, 2, 0, 3) \
            .reshape(MC, B)                             # [MC(station-major), B]
        lo_ = MC * core
        hi_ = min(MC * (core + 1), n_users)
        out[:, lo_:hi_] = cols[:hi_ - lo_].T

    # exact host recomputation of the <=2*B*K beta/scatter-affected entries
    for b in range(B):
        now_b = now_nodes[b]
        his_b = his_nodes[b]
        aff = np.unique(np.concatenate([now_b, his_b]))
        pos_now = np.searchsorted(aff, now_b)
        pos_his = np.searchsorted(aff, his_b)

        proj_aff = station_emb_table[aff] @ proj_w + proj_b     # [m, D]
        embed = (1.0 - alpha[aff]) * proj_aff                   # beta == 1
        now_add = alpha[now_b] * station_embedding[now_b]       # [K, D]
        np.add.at(embed, pos_now, now_add)
        h = _leaky_relu(raw_field_embed[his_b] @ w_his1 + b_his1) \
            @ w_his2 + b_his2                                   # [K, D]
        np.add.at(embed, pos_his, alpha[his_b] * h)
        out[b, aff] = embed @ w_st + b_st[0] + user_out[b]

    return out


# revision 20
# speedup vs baseline: 1.1293x; 1.1213x over previous
"""Trainium2 Bass kernel for nn_PredictionDecoder.

Factorization: the reference materializes embed [B,N,D] but immediately
contracts it with w_st, so

  out[b,n] = (1 - beta[b,n]*alpha[n]) * s[n]
             + scatter corrections at <= 2*B*K positions
             + user_out[b] + b_st
  with s[n] = (station_emb_table[n] @ proj_w + proj_b) @ w_st
            = station_emb_table[n] @ v + c,   v = proj_w @ w_st

The memory-bound part is the [N,D] @ [D] matvec (streams the 30.8 MB
table).  N is sharded over the 8 cores.  The fp32 table is shipped as a
bf16 hi/lo split (same bytes, ~2^-18 relative precision) because bf16
matmuls get fast weight load; each 128-row chunk is the stationary
operand and v (replicated x8 for the batch dim, hi and lo) is the moving
operand, so PSUM directly holds the [n_part, b] output layout.  The
vector engine sums the three product terms plus the per-b bias, and the
[7680, 8] output slice leaves with fully-contiguous DMA.  The <=512
beta/scatter-affected (b, n) entries are recomputed exactly on the host
and patched in.
"""

import os
import numpy as np

B = 8
D = 128
N_USER = 60082
NCORES = 8
MC = 7680                 # table rows per core (padded)
NPAD = NCORES * MC        # 61440
TBLK = 2560               # columns per DMA block
NBLK = MC // TBLK         # 3
KPB = TBLK // 128         # 20 matmul chunks per block
NCHUNK = MC // 128        # 60
FREE = NCHUNK * B         # 480 output columns
PFREE = NCHUNK * 24       # 1440 psum columns (16 hi-terms + 8 lo-term)

_CACHE = {}
LAST_RESULT = None


def _build_program():
    import concourse.tile as tile
    from concourse import bacc, mybir

    nc = bacc.Bacc("TRN2", target_bir_lowering=False, debug=False,
                   num_devices=NCORES)
    f32 = mybir.dt.float32
    bf16 = mybir.dt.bfloat16
    tabhi = nc.dram_tensor("tabhi", [128, MC], bf16, kind="ExternalInput")
    tablo = nc.dram_tensor("tablo", [128, MC], bf16, kind="ExternalInput")
    v16 = nc.dram_tensor("v16", [128, 2 * B], bf16, kind="ExternalInput")
    urep = nc.dram_tensor("urep", [128, B], f32, kind="ExternalInput")
    res = nc.dram_tensor("res", [MC, B], f32, kind="ExternalOutput")

    with tile.TileContext(nc) as tc:
        with tc.tile_pool(name="hi", bufs=NBLK) as hi_pool, \
             tc.tile_pool(name="lo", bufs=NBLK) as lo_pool, \
             tc.tile_pool(name="small", bufs=1) as small_pool, \
             tc.tile_pool(name="outp", bufs=1) as out_pool, \
             tc.tile_pool(name="ps", bufs=1, space="PSUM") as ps_pool:
            v16_sb = small_pool.tile([128, 2 * B], bf16, tag="v16")
            nc.gpsimd.dma_start(v16_sb[:], v16.ap())
            u_sb = small_pool.tile([128, B], f32, tag="u")
            nc.gpsimd.dma_start(u_sb[:], urep.ap())

            out_sb = out_pool.tile([128, FREE], f32, tag="o")
            tmp1 = out_pool.tile([128, FREE], f32, tag="t1")
            tmp2 = out_pool.tile([128, FREE], f32, tag="t2")

            # warm-up matmul consuming only v16 so the PE observes the v16 DMA
            # semaphore before the table matmuls begin.
            scr = ps_pool.tile([2 * B, 2 * B], f32, tag="scr")
            nc.tensor.matmul(out=scr[:], lhsT=v16_sb[:], rhs=v16_sb[:],
                             start=True, stop=True)

            # three data rings; gpsimd starts earliest, scalar latest
            hi_ring = [nc.sync, nc.gpsimd, nc.scalar]
            lo_ring = [nc.scalar, nc.gpsimd, nc.sync]

            o3 = out_sb[:].rearrange("p (k c) -> p k c", c=B)
            t13 = tmp1[:].rearrange("p (k c) -> p k c", c=B)
            t23 = tmp2[:].rearrange("p (k c) -> p k c", c=B)
            u3 = u_sb[:].unsqueeze(1).broadcast_to([128, KPB, B])
            resv = res.ap().rearrange("(p q) b -> p (q b)", p=128)

            for t in range(NBLK):
                sl = slice(TBLK * t, TBLK * (t + 1))
                hi_blk = hi_pool.tile([128, TBLK], bf16, tag="hi")
                hi_ring[t].dma_start(hi_blk[:], tabhi.ap()[:, sl])
                lo_blk = lo_pool.tile([128, TBLK], bf16, tag="lo")
                lo_ring[t].dma_start(lo_blk[:], tablo.ap()[:, sl])
                # per-block PSUM tiles give the scheduler exact dependencies
                psh = ps_pool.tile([128, KPB * 16], f32, tag=f"ph{t}")
                psl = ps_pool.tile([128, KPB * 8], f32, tag=f"pl{t}")
                for kk in range(KPB):
                    wsl = slice(128 * kk, 128 * (kk + 1))
                    # hi @ [v_hi | v_lo] -> 16 cols
                    nc.tensor.matmul(
                        out=psh[:, 16 * kk:16 * kk + 16],
                        lhsT=hi_blk[:, wsl], rhs=v16_sb[:],
                        start=True, stop=True)
                    # lo @ v_hi -> 8 cols
                    nc.tensor.matmul(
                        out=psl[:, 8 * kk:8 * kk + 8],
                        lhsT=lo_blk[:, wsl], rhs=v16_sb[:, 0:B],
                        start=True, stop=True)

                pv = psh[:].rearrange("p (k c) -> p k c", c=16)
                ck = slice(KPB * t, KPB * (t + 1))
                fs = slice(KPB * B * t, KPB * B * (t + 1))
                # DVE may read only one PSUM operand per instruction
                nc.vector.tensor_add(t13[:, ck], pv[:, :, 0:8], u3)
                nc.vector.tensor_add(t23[:, ck], t13[:, ck], pv[:, :, 8:16])
                nc.vector.tensor_add(
                    o3[:, ck], t23[:, ck],
                    psl[:].rearrange("p (k c) -> p k c", c=B))
                nc.sync.dma_start(resv[:, fs], out_sb[:, fs])
    nc.compile()
    return nc


def _get_program():
    if "nc" not in _CACHE:
        _CACHE["nc"] = _build_program()
    return _CACHE["nc"]


def _leaky_relu(x):
    return np.where(x >= 0, x, 0.01 * x)


def _hi_lo(x, bf16):
    hi = x.astype(bf16)
    lo = (x - hi.astype(np.float32)).astype(bf16)
    return hi, lo


def kernel(user_embedding, station_embedding, raw_field_embed,
           user_emb_table, station_emb_table, proj_w, proj_b,
           theta, alpha, w_his1, b_his1, w_his2, b_his2,
           w_st, b_st, w_u, b_u,
           his_nodes, now_nodes, user_id):
    import ml_dtypes
    from concourse.bass_utils import run_bass_kernel_spmd
    global LAST_RESULT

    f32 = np.float32
    bf16 = ml_dtypes.bfloat16
    user_embedding = np.asarray(user_embedding, f32)
    station_embedding = np.asarray(station_embedding, f32)
    raw_field_embed = np.asarray(raw_field_embed, f32)
    user_emb_table = np.asarray(user_emb_table, f32)
    station_emb_table = np.asarray(station_emb_table, f32)
    proj_w = np.asarray(proj_w, f32)
    proj_b = np.asarray(proj_b, f32).reshape(-1)
    theta = np.asarray(theta, f32)
    alpha = np.asarray(alpha, f32)
    w_his1 = np.asarray(w_his1, f32)
    b_his1 = np.asarray(b_his1, f32).reshape(-1)
    w_his2 = np.asarray(w_his2, f32)
    b_his2 = np.asarray(b_his2, f32).reshape(-1)
    w_st = np.asarray(w_st, f32).reshape(-1)
    b_st = np.asarray(b_st, f32).reshape(-1)
    w_u = np.asarray(w_u, f32).reshape(-1)
    b_u = np.asarray(b_u, f32).reshape(-1)
    his_nodes = np.asarray(his_nodes).astype(np.int64)
    now_nodes = np.asarray(now_nodes).astype(np.int64)
    user_id = np.asarray(user_id).astype(np.int64)

    n_users = station_emb_table.shape[0]

    # host-side small linear algebra (exact, matches reference math)
    v = proj_w @ w_st                                   # [D]
    c = f32(proj_b @ w_st)                              # scalar
    th = theta[user_id, 0]                              # [B]
    user_mem = (1.0 - th)[:, None] * user_embedding \
        + th[:, None] * user_emb_table[user_id]         # [B, D]
    user_out = user_mem @ w_u + b_u[0]                  # [B]
    bias = (user_out + b_st[0] + c).astype(f32)         # [B]

    # device inputs
    tab_pad = np.zeros((NPAD, D), f32)
    tab_pad[:n_users] = station_emb_table
    v_hi, v_lo = _hi_lo(v.astype(f32), bf16)
    v16 = np.empty((D, 2 * B), bf16)
    v16[:, 0:B] = v_hi[:, None]
    v16[:, B:2 * B] = v_lo[:, None]
    urep = np.ascontiguousarray(np.broadcast_to(bias[None, :], (128, B)))

    in_maps = []
    for core in range(NCORES):
        shard = np.ascontiguousarray(
            tab_pad[MC * core:MC * (core + 1)].T)       # [128, MC]
        hi, lo = _hi_lo(shard, bf16)
        in_maps.append({"tabhi": hi, "tablo": lo, "v16": v16, "urep": urep})

    nc = _get_program()
    trace = bool(int(os.environ.get("KERNEL_TRACE", "0")))
    LAST_RESULT = run_bass_kernel_spmd(
        nc, in_maps, core_ids=list(range(NCORES)), trace=trace)

    # unshard: res[60*j + 6*t + kk, b] = s[off + 768*t + 128*kk + j] + bias[b]
    out = np.empty((B, N_USER), f32)
    for core in range(NCORES):
        res = LAST_RESULT.results[core]["res"]          # [MC, B]
        cols = res.reshape(128, NBLK, KPB, B).transpose(1, 2, 0, 3) \
            .reshape(MC, B)                             # [MC(station-major), B]
        lo_ = MC * core
        hi_ = min(MC * (core + 1), n_users)
        out[:, lo_:hi_] = cols[:hi_ - lo_].T

    # exact host recomputation of the <=2*B*K beta/scatter-affected entries
    for b in range(B):
        now_b = now_nodes[b]
        his_b = his_nodes[b]
        aff = np.unique(np.concatenate([now_b, his_b]))
        pos_now = np.searchsorted(aff, now_b)
        pos_his = np.searchsorted(aff, his_b)

        proj_aff = station_emb_table[aff] @ proj_w + proj_b     # [m, D]
        embed = (1.0 - alpha[aff]) * proj_aff                   # beta == 1
        now_add = alpha[now_b] * station_embedding[now_b]       # [K, D]
        np.add.at(embed, pos_now, now_add)
        h = _leaky_relu(raw_field_embed[his_b] @ w_his1 + b_his1) \
            @ w_his2 + b_his2                                   # [K, D]
        np.add.at(embed, pos_his, alpha[his_b] * h)
        out[b, aff] = embed @ w_st + b_st[0] + user_out[b]

    return out


# revision 21
# speedup vs baseline: 1.3785x; 1.2207x over previous
"""Trainium2 Bass kernel for nn_PredictionDecoder.

Factorization: the reference materializes embed [B,N,D] but immediately
contracts it with w_st, so

  out[b,n] = (1 - beta[b,n]*alpha[n]) * s[n]
             + scatter corrections at <= 2*B*K positions
             + user_out[b] + b_st
  with s[n] = (station_emb_table[n] @ proj_w + proj_b) @ w_st
            = station_emb_table[n] @ v + c,   v = proj_w @ w_st

The memory-bound part is the [N,D] @ [D] matvec (streams the table).  N
is sharded over the 8 cores.  The table is shipped fp16 (half the DMA
bytes; the table values are O(0.02) so fp16 rounding contributes ~1e-5
relative error); v stays accurate via an fp16 hi/lo split ([v_hi |
v_lo*2^10], descaled on the vector engine).  Each 128-row chunk is the
stationary operand and the 16 v columns are the moving operand, so PSUM
directly holds the [n_part, b] output layout.  Input blocks stream over
both HWDGE rings; per-block PSUM tiles let the adds and the output DMA
of early blocks overlap later blocks' input.  The <=512
beta/scatter-affected (b, n) entries are recomputed exactly on the host
and patched in.
"""

import os
import numpy as np

B = 8
D = 128
N_USER = 60082
NCORES = 8
MC = 7680                 # table rows per core (padded)
NPAD = NCORES * MC        # 61440
TBLK = 2560               # columns per DMA block
NBLK = MC // TBLK         # 3
KPB = TBLK // 128         # 20 matmul chunks per block
NCHUNK = MC // 128        # 60
FREE = NCHUNK * B         # 480 output columns
VLO_SCALE = 1024.0        # keeps v_lo out of fp16 subnormal range

_CACHE = {}
LAST_RESULT = None


def _build_program():
    import concourse.tile as tile
    from concourse import bacc, mybir

    nc = bacc.Bacc("TRN2", target_bir_lowering=False, debug=False,
                   num_devices=NCORES)
    f32 = mybir.dt.float32
    f16 = mybir.dt.float16
    tab16 = nc.dram_tensor("tab16", [128, MC], f16, kind="ExternalInput")
    v16 = nc.dram_tensor("v16", [128, 2 * B], f16, kind="ExternalInput")
    urep = nc.dram_tensor("urep", [128, B], f32, kind="ExternalInput")
    res = nc.dram_tensor("res", [MC, B], f32, kind="ExternalOutput")

    with tile.TileContext(nc) as tc:
        with tc.tile_pool(name="tab", bufs=NBLK) as tab_pool, \
             tc.tile_pool(name="small", bufs=1) as small_pool, \
             tc.tile_pool(name="outp", bufs=1) as out_pool, \
             tc.tile_pool(name="ps", bufs=1, space="PSUM") as ps_pool:
            v16_sb = small_pool.tile([128, 2 * B], f16, tag="v16")
            nc.scalar.dma_start(v16_sb[:], v16.ap())
            u_sb = small_pool.tile([128, B], f32, tag="u")
            nc.scalar.dma_start(u_sb[:], urep.ap())

            out_sb = out_pool.tile([128, FREE], f32, tag="o")
            tmp1 = out_pool.tile([128, FREE], f32, tag="t1")
            tmp2 = out_pool.tile([128, FREE], f32, tag="t2")

            # warm-up matmul consuming only v16 so the PE observes the v16 DMA
            # semaphore before the table matmuls begin.
            scr = ps_pool.tile([2 * B, 2 * B], f32, tag="scr")
            nc.tensor.matmul(out=scr[:], lhsT=v16_sb[:], rhs=v16_sb[:],
                             start=True, stop=True)

            ring = [nc.sync, nc.scalar, nc.sync]

            o3 = out_sb[:].rearrange("p (k c) -> p k c", c=B)
            t13 = tmp1[:].rearrange("p (k c) -> p k c", c=B)
            t23 = tmp2[:].rearrange("p (k c) -> p k c", c=B)
            u3 = u_sb[:].unsqueeze(1).broadcast_to([128, KPB, B])
            resv = res.ap().rearrange("(p q) b -> p (q b)", p=128)

            for t in range(NBLK):
                sl = slice(TBLK * t, TBLK * (t + 1))
                blk = tab_pool.tile([128, TBLK], f16, tag="tab")
                ring[t].dma_start(blk[:], tab16.ap()[:, sl])
                # per-block PSUM tiles give the scheduler exact dependencies
                psh = ps_pool.tile([128, KPB * 16], f32, tag=f"ph{t}")
                for kk in range(KPB):
                    wsl = slice(128 * kk, 128 * (kk + 1))
                    nc.tensor.matmul(
                        out=psh[:, 16 * kk:16 * kk + 16],
                        lhsT=blk[:, wsl], rhs=v16_sb[:],
                        start=True, stop=True)

                pv = psh[:].rearrange("p (k c) -> p k c", c=16)
                ck = slice(KPB * t, KPB * (t + 1))
                fs = slice(KPB * B * t, KPB * B * (t + 1))
                # DVE may read only one PSUM operand per instruction
                nc.vector.tensor_add(t13[:, ck], pv[:, :, 0:8], u3)
                nc.vector.tensor_scalar_mul(t23[:, ck], pv[:, :, 8:16],
                                            1.0 / VLO_SCALE)
                nc.vector.tensor_add(o3[:, ck], t13[:, ck], t23[:, ck])
                nc.sync.dma_start(resv[:, fs], out_sb[:, fs])
    nc.compile()
    return nc


def _get_program():
    if "nc" not in _CACHE:
        _CACHE["nc"] = _build_program()
    return _CACHE["nc"]


def _leaky_relu(x):
    return np.where(x >= 0, x, 0.01 * x)


def kernel(user_embedding, station_embedding, raw_field_embed,
           user_emb_table, station_emb_table, proj_w, proj_b,
           theta, alpha, w_his1, b_his1, w_his2, b_his2,
           w_st, b_st, w_u, b_u,
           his_nodes, now_nodes, user_id):
    from concourse.bass_utils import run_bass_kernel_spmd
    global LAST_RESULT

    f32 = np.float32
    f16 = np.float16
    user_embedding = np.asarray(user_embedding, f32)
    station_embedding = np.asarray(station_embedding, f32)
    raw_field_embed = np.asarray(raw_field_embed, f32)
    user_emb_table = np.asarray(user_emb_table, f32)
    station_emb_table = np.asarray(station_emb_table, f32)
    proj_w = np.asarray(proj_w, f32)
    proj_b = np.asarray(proj_b, f32).reshape(-1)
    theta = np.asarray(theta, f32)
    alpha = np.asarray(alpha, f32)
    w_his1 = np.asarray(w_his1, f32)
    b_his1 = np.asarray(b_his1, f32).reshape(-1)
    w_his2 = np.asarray(w_his2, f32)
    b_his2 = np.asarray(b_his2, f32).reshape(-1)
    w_st = np.asarray(w_st, f32).reshape(-1)
    b_st = np.asarray(b_st, f32).reshape(-1)
    w_u = np.asarray(w_u, f32).reshape(-1)
    b_u = np.asarray(b_u, f32).reshape(-1)
    his_nodes = np.asarray(his_nodes).astype(np.int64)
    now_nodes = np.asarray(now_nodes).astype(np.int64)
    user_id = np.asarray(user_id).astype(np.int64)

    n_users = station_emb_table.shape[0]

    # host-side small linear algebra (exact, matches reference math)
    v = proj_w @ w_st                                   # [D]
    c = f32(proj_b @ w_st)                              # scalar
    th = theta[user_id, 0]                              # [B]
    user_mem = (1.0 - th)[:, None] * user_embedding \
        + th[:, None] * user_emb_table[user_id]         # [B, D]
    user_out = user_mem @ w_u + b_u[0]                  # [B]
    bias = (user_out + b_st[0] + c).astype(f32)         # [B]

    # device inputs
    tab_pad = np.zeros((NPAD, D), f32)
    tab_pad[:n_users] = station_emb_table
    v_hi = v.astype(f16)
    v_lo = ((v - v_hi.astype(f32)) * VLO_SCALE).astype(f16)
    v16 = np.empty((D, 2 * B), f16)
    v16[:, 0:B] = v_hi[:, None]
    v16[:, B:2 * B] = v_lo[:, None]
    urep = np.ascontiguousarray(np.broadcast_to(bias[None, :], (128, B)))

    in_maps = []
    for core in range(NCORES):
        shard = np.ascontiguousarray(
            tab_pad[MC * core:MC * (core + 1)].T)       # [128, MC]
        in_maps.append({"tab16": shard.astype(f16), "v16": v16, "urep": urep})

    nc = _get_program()
    trace = bool(int(os.environ.get("KERNEL_TRACE", "0")))
    LAST_RESULT = run_bass_kernel_spmd(
        nc, in_maps, core_ids=list(range(NCORES)), trace=trace)

    # unshard: res[60*j + KPB*t + kk, b] = s[off + 2560*t + 128*kk + j] + bias[b]
    out = np.empty((B, N_USER), f32)
    for core in range(NCORES):
        res = LAST_RESULT.results[core]["res"]          # [MC, B]
        cols = res.reshape(128, NBLK, KPB, B).transpose(1, 2, 0, 3) \
            .reshape(MC, B)                             # [MC(station-major), B]
        lo_ = MC * core
        hi_ = min(MC * (core + 1), n_users)
        out[:, lo_:hi_] = cols[:hi_ - lo_].T

    # exact host recomputation of the <=2*B*K beta/scatter-affected entries
    for b in range(B):
        now_b = now_nodes[b]
        his_b = his_nodes[b]
        aff = np.unique(np.concatenate([now_b, his_b]))
        pos_now = np.searchsorted(aff, now_b)
        pos_his = np.searchsorted(aff, his_b)

        proj_aff = station_emb_table[aff] @ proj_w + proj_b     # [m, D]
        embed = (1.0 - alpha[aff]) * proj_aff                   # beta == 1
        now_add = alpha[now_b] * station_embedding[now_b]       # [K, D]
        np.add.at(embed, pos_now, now_add)
        h = _leaky_relu(raw_field_embed[his_b] @ w_his1 + b_his1) \
            @ w_his2 + b_his2                                   # [K, D]
        np.add.at(embed, pos_his, alpha[his_b] * h)
        out[b, aff] = embed @ w_st + b_st[0] + user_out[b]

    return out
